# revision 3
# baseline (speedup 1.0000x reference)
"""Channel-attention block (AttentionBlock, C=64) on 8 trn2 NeuronCores.

Algebraic reduction: with q = wq x + bq etc. and attention over channels,
    S  = q k^T / sqrt(C) = wqa^T_aug G_aug wka_aug / 8,   G_aug = [[x x^T, s],[s^T, N]]
    out = softmax(S) v + x = (attn wv + I) x + (attn bv) 1^T
so the kernel only needs the 65x65 Gram (per batch) of x plus two passes of
tiny channel matmuls over x.  N axis is sharded over 8 cores; the [G|s]
partial sums (66 KB) are all-reduced on-device.

Layout: both batches stacked on the partition axis (p = b*64 + c), so every
matmul runs with full K=M=128 via block-diagonal weights.
"""

import numpy as np

import concourse.bacc as bacc
import concourse.bass as bass
import concourse.mybir as mybir
import concourse.tile as tile
from concourse import bass_utils

F32 = mybir.dt.float32

NCORES = 8
B, C = 2, 64
P = B * C  # 128 partitions, batches stacked
N_TOTAL = 64 * 64 * 64  # 262144
N_SHARD = N_TOTAL // NCORES  # 32768
GCHUNK = 128  # transpose/gram chunk (PE transpose is 128x128)
N_GCH = N_SHARD // GCHUNK  # 256
OCHUNK = 512  # phase-2 matmul free dim (fp32 max)
N_OCH = N_SHARD // OCHUNK  # 64
LDCHUNK = 2048  # input DMA slice
N_LDCH = N_SHARD // LDCHUNK


def build_bass():
    nc = bacc.Bacc(
        "TRN2",
        target_bir_lowering=False,
        debug=False,
        num_devices=NCORES,
    )

    x_t = nc.dram_tensor("x", [P, N_SHARD], F32, kind="ExternalInput")
    wqa_t = nc.dram_tensor("wqa", [65, 64], F32, kind="ExternalInput")  # [wq|bq]^T/8
    wka_t = nc.dram_tensor("wka", [65, 64], F32, kind="ExternalInput")  # [wk|bk]^T
    wv_t = nc.dram_tensor("wv", [64, 64], F32, kind="ExternalInput")
    bv_t = nc.dram_tensor("bv", [64, 1], F32, kind="ExternalInput")
    id_t = nc.dram_tensor("ident", [128, 128], F32, kind="ExternalInput")
    out_t = nc.dram_tensor("out", [P, N_SHARD], F32, kind="ExternalOutput")

    with tile.TileContext(nc, num_cores=NCORES) as tc:
        with (
            tc.tile_pool(name="xbuf", bufs=1) as xpool,
            tc.tile_pool(name="consts", bufs=1) as cpool,
            tc.tile_pool(name="xt", bufs=6) as xtpool,
            tc.tile_pool(name="osb", bufs=6) as opool,
            tc.tile_pool(name="gacc", bufs=1, space="PSUM") as gpool,
            tc.tile_pool(name="tps", bufs=2, space="PSUM") as tpool,
            tc.tile_pool(name="pmath", bufs=1, space="PSUM") as mpool,
            tc.tile_pool(name="ops", bufs=2, space="PSUM") as oppool,
            tc.tile_pool(name="dram", bufs=2, space="DRAM") as dram,
        ):
            # ---- constants to SBUF ----
            ident = cpool.tile([128, 128], F32)
            nc.sync.dma_start(ident[:], id_t[:, :])
            wqa = cpool.tile([65, 64], F32)
            nc.sync.dma_start(wqa[:], wqa_t[:, :])
            wka = cpool.tile([65, 64], F32)
            nc.sync.dma_start(wka[:], wka_t[:, :])
            wv = cpool.tile([64, 64], F32)
            nc.sync.dma_start(wv[:], wv_t[:, :])
            bv = cpool.tile([64, 1], F32)
            nc.sync.dma_start(bv[:], bv_t[:, :])

            # ---- load x shard (persistent in SBUF; subtile deps let compute
            # start as each slice lands) ----
            xs = xpool.tile([P, N_SHARD], F32)
            for k in range(N_LDCH):
                sl = slice(k * LDCHUNK, (k + 1) * LDCHUNK)
                nc.sync.dma_start(xs[:, sl], x_t[:, sl])

            # ---- phase 1: G_psum[:, 0:128] += xT^T xT ; col 128 = row sums ----
            g_ps = gpool.tile([P, 129], F32)
            for j in range(N_GCH):
                sl = slice(j * GCHUNK, (j + 1) * GCHUNK)
                tps = tpool.tile([128, GCHUNK], F32)
                nc.tensor.transpose(tps[:], xs[:, sl], ident[:])
                xt = xtpool.tile([128, 129], F32)
                nc.gpsimd.memset(xt[:, 128:129], 1.0)
                if j % 2 == 0:
                    nc.vector.tensor_copy(xt[:, 0:128], tps[:])
                else:
                    nc.scalar.copy(xt[:, 0:128], tps[:])
                nc.tensor.matmul(
                    g_ps[:],
                    lhsT=xt[:, 0:128],
                    rhs=xt[:, 0:129],
                    start=(j == 0),
                    stop=(j == N_GCH - 1),
                )

            gs = cpool.tile([P, 129], F32)
            nc.vector.tensor_copy(gs[:], g_ps[:])

            # ---- all-reduce [G|s] over the 8 cores (HBM bounce) ----
            cc_in = dram.tile([P, 129], F32)
            cc_out = dram.tile([P, 129], F32)
            nc.sync.dma_start(cc_in, gs[:])
            nc.gpsimd.collective_compute(
                "AllReduce",
                mybir.AluOpType.add,
                replica_groups=[list(range(NCORES))],
                ins=[cc_in.opt()],
                outs=[cc_out.opt()],
            )
            gsr = cpool.tile([P, 129], F32)
            nc.sync.dma_start(gsr[:], cc_out)

            # ---- tiny math: S = wqa^T G_aug wka ; softmax ; QT, c ----
            # s^T row via PE transpose of the s column
            st_ps = mpool.tile([1, 128], F32, tag="m1")
            nc.tensor.transpose(st_ps[:], gsr[:, 128:129], ident[:])
            st = cpool.tile([1, 128], F32)
            nc.vector.tensor_copy(st[:], st_ps[:])

            ga = []
            for b in range(B):
                g_aug = cpool.tile([65, 65], F32, tag=f"ga{b}", name=f"g_aug{b}")
                cs = slice(b * 64, (b + 1) * 64)
                nc.vector.tensor_copy(g_aug[0:64, 0:64], gsr[cs, cs])
                nc.vector.tensor_copy(g_aug[0:64, 64:65], gsr[cs, 128:129])
                nc.vector.tensor_copy(g_aug[64:65, 0:64], st[:, cs])
                nc.vector.memset(g_aug[64:65, 64:65], float(N_TOTAL))
                ga.append(g_aug)

            # A_b = G_aug_b @ wka  (G_aug symmetric -> lhsT = G_aug)
            s_ps = mpool.tile([P, 64], F32, tag="m2")
            for b in range(B):
                a_ps = mpool.tile([65, 64], F32, tag="m1", name=f"a_ps{b}")
                nc.tensor.matmul(a_ps[:], lhsT=ga[b][:], rhs=wka[:])
                a_sb = cpool.tile([65, 64], F32, tag=f"asb{b}", name=f"a_sb{b}")
                nc.vector.tensor_copy(a_sb[:], a_ps[:])
                # S_b = wqa^T @ A_b   (scale 1/8 folded into wqa)
                nc.tensor.matmul(
                    s_ps[b * 64 : (b + 1) * 64, :], lhsT=wqa[:], rhs=a_sb[:]
                )

            # softmax rows (both batches stacked [128, 64])
            negmax = cpool.tile([P, 1], F32)
            nc.vector.reduce_max(negmax[:], s_ps[:], axis=mybir.AxisListType.X,
                                 negate=True)
            expv = cpool.tile([P, 64], F32)
            rowsum = cpool.tile([P, 1], F32)
            nc.scalar.activation(
                expv[:], s_ps[:], mybir.ActivationFunctionType.Exp,
                bias=negmax[:, 0:1], scale=1.0, accum_out=rowsum[:, 0:1],
            )
            rinv = cpool.tile([P, 1], F32)
            nc.vector.reciprocal(rinv[:], rowsum[:])
            attn = cpool.tile([P, 64], F32)
            nc.vector.tensor_scalar_mul(attn[:], expv[:], rinv[:, 0:1])

            # attn^T (one transpose: [128,64] -> [64,128] = [attn0^T | attn1^T])
            at_ps = mpool.tile([64, 128], F32, tag="m1")
            nc.tensor.transpose(at_ps[:], attn[:], ident[:])
            at_sb = cpool.tile([64, 128], F32)
            nc.vector.tensor_copy(at_sb[:], at_ps[:])

            # QT block-diag [128,128]: QT_b = wv^T attn_b^T + I
            qt_ps = mpool.tile([128, 128], F32, tag="m2")
            c_ps = mpool.tile([128, 1], F32, tag="m3")
            for b in range(B):
                cs = slice(b * 64, (b + 1) * 64)
                nc.tensor.matmul(
                    qt_ps[cs, cs], lhsT=wv[:], rhs=at_sb[:, cs],
                    start=True, stop=False,
                )
                nc.tensor.matmul(
                    qt_ps[cs, cs], lhsT=ident[0:64, 0:64], rhs=ident[0:64, 0:64],
                    start=False, stop=True,
                )
                nc.tensor.matmul(c_ps[cs, :], lhsT=at_sb[:, cs], rhs=bv[:])
            qt = cpool.tile([128, 128], F32)
            nc.vector.memset(qt[:], 0.0)
            for b in range(B):
                cs = slice(b * 64, (b + 1) * 64)
                nc.vector.tensor_copy(qt[cs, cs], qt_ps[cs, cs])
            cvec = cpool.tile([P, 1], F32)
            nc.vector.tensor_copy(cvec[:], c_ps[:])

            # ---- phase 2: out = QT^T x + c ----
            for k in range(N_OCH):
                sl = slice(k * OCHUNK, (k + 1) * OCHUNK)
                o_ps = oppool.tile([P, OCHUNK], F32)
                nc.tensor.matmul(o_ps[:], lhsT=qt[:], rhs=xs[:, sl])
                osb = opool.tile([P, OCHUNK], F32)
                if k % 2 == 0:
                    nc.vector.tensor_scalar_add(osb[:], o_ps[:], cvec[:, 0:1])
                else:
                    nc.scalar.add(osb[:], o_ps[:], cvec[:, 0:1])
                nc.sync.dma_start(out_t[:, sl], osb[:])

    nc.compile()
    return nc


_cached_nc = None


def kernel(x, wq, bq, wk, bk, wv, bv, _trace=False):
    global _cached_nc
    x = np.ascontiguousarray(np.asarray(x, dtype=np.float32))
    assert x.shape == (B, C, 64, 64, 64)
    xf = x.reshape(P, N_TOTAL)

    wqa = (
        np.concatenate(
            [np.asarray(wq, np.float64), np.asarray(bq, np.float64)[:, None]], axis=1
        ).T
        / 8.0
    ).astype(np.float32)  # [65, 64]
    wka = (
        np.concatenate(
            [np.asarray(wk, np.float64), np.asarray(bk, np.float64)[:, None]], axis=1
        ).T
    ).astype(np.float32)  # [65, 64]
    wv32 = np.ascontiguousarray(np.asarray(wv, np.float32))
    bv32 = np.ascontiguousarray(np.asarray(bv, np.float32).reshape(64, 1))
    ident = np.eye(128, dtype=np.float32)

    in_maps = []
    for i in range(NCORES):
        sl = slice(i * N_SHARD, (i + 1) * N_SHARD)
        in_maps.append(
            {
                "x": np.ascontiguousarray(xf[:, sl]),
                "wqa": wqa,
                "wka": wka,
                "wv": wv32,
                "bv": bv32,
                "ident": ident,
            }
        )

    if _cached_nc is None:
        _cached_nc = build_bass()
    nc = _cached_nc

    res = bass_utils.run_bass_kernel_spmd(
        nc, in_maps, core_ids=list(range(NCORES)), trace=_trace
    )
    kernel._last_results = res

    out = np.empty((P, N_TOTAL), dtype=np.float32)
    for i in range(NCORES):
        out[:, i * N_SHARD : (i + 1) * N_SHARD] = res.results[i]["out"]
    return out.reshape(B, C, 64, 64, 64)


kernel._last_results = None


# revision 10
# speedup vs baseline: 1.3065x; 1.3065x over previous
"""Channel-attention block (AttentionBlock, C=64) on 8 trn2 NeuronCores.

Algebraic reduction: with q = wq x + bq etc. and attention over channels,
    S  = q k^T / sqrt(C) = wqa^T_aug G_aug wka_aug / 8,   G_aug = [[x x^T, s],[s^T, N]]
    out = softmax(S) v + x = (attn wv + I) x + (attn bv) 1^T
so the kernel only needs the 65x65 Gram (per batch) of x plus two passes of
tiny channel matmuls over x.  N axis is sharded over 8 cores; the [G|s]
partial sums (66 KB) are all-reduced on-device.

Layout: both batches stacked on the partition axis (p = b*64 + c), so every
matmul runs with full K=M=128 via block-diagonal weights.
"""

import numpy as np

import concourse.bacc as bacc
import concourse.bass as bass
import concourse.mybir as mybir
import concourse.tile as tile
from concourse import bass_utils

F32 = mybir.dt.float32

NCORES = 8
B, C = 2, 64
P = B * C  # 128 partitions, batches stacked
N_TOTAL = 64 * 64 * 64  # 262144
N_SHARD = N_TOTAL // NCORES  # 32768
GCHUNK = 128  # transpose/gram chunk (PE transpose is 128x128)
N_GCH = N_SHARD // GCHUNK  # 256
OCHUNK = 512  # phase-2 matmul free dim (fp32 max)
N_OCH = N_SHARD // OCHUNK  # 64
LDCHUNK = 2048  # input DMA slice
N_LDCH = N_SHARD // LDCHUNK


def build_bass():
    nc = bacc.Bacc(
        "TRN2",
        target_bir_lowering=False,
        debug=False,
        num_devices=NCORES,
    )

    x_t = nc.dram_tensor("x", [P, N_SHARD], F32, kind="ExternalInput")
    wqa_t = nc.dram_tensor("wqa", [65, 64], F32, kind="ExternalInput")  # [wq|bq]^T/8
    wka_t = nc.dram_tensor("wka", [65, 64], F32, kind="ExternalInput")  # [wk|bk]^T
    wv_t = nc.dram_tensor("wv", [64, 64], F32, kind="ExternalInput")
    bv_t = nc.dram_tensor("bv", [64, 1], F32, kind="ExternalInput")
    id_t = nc.dram_tensor("ident", [128, 128], F32, kind="ExternalInput")
    out_t = nc.dram_tensor("out", [P, N_SHARD], F32, kind="ExternalOutput")

    with tile.TileContext(nc, num_cores=NCORES) as tc:
        with (
            tc.tile_pool(name="xbuf", bufs=1) as xpool,
            tc.tile_pool(name="consts", bufs=1) as cpool,
            tc.tile_pool(name="xt", bufs=8) as xtpool,
            tc.tile_pool(name="osb", bufs=8) as opool,
            tc.tile_pool(name="dram", bufs=2, space="DRAM") as dram,
        ):
            # ---- constants to SBUF ----
            ident = cpool.tile([128, 128], F32)
            nc.sync.dma_start(ident[:], id_t[:, :])
            wqa = cpool.tile([65, 64], F32)
            nc.sync.dma_start(wqa[:], wqa_t[:, :])
            wka = cpool.tile([65, 64], F32)
            nc.sync.dma_start(wka[:], wka_t[:, :])
            wv = cpool.tile([64, 64], F32)
            nc.sync.dma_start(wv[:], wv_t[:, :])
            bv = cpool.tile([64, 1], F32)
            nc.sync.dma_start(bv[:], bv_t[:, :])

            # ---- load x shard (persistent in SBUF; subtile deps let compute
            # start as each slice lands) ----
            xs = xpool.tile([P, N_SHARD], F32)
            for k in range(N_LDCH):
                sl = slice(k * LDCHUNK, (k + 1) * LDCHUNK)
                nc.sync.dma_start(xs[:, sl], x_t[:, sl])

            # ---- phase 1: G_psum[:, 0:128] += xT^T xT ; cols 128/129 = row sums ----
            # fp32r => single-pass matmul (~tf32 precision; verified ample).
            # fp32r needs even free-dim counts -> two ones columns (s duplicated).
            F32R = mybir.dt.float32r
            ones_f = cpool.tile([128, 2], F32)
            nc.vector.memset(ones_f[:], 1.0)
            ones_r = cpool.tile([128, 2], F32R)
            nc.vector.tensor_copy(ones_r[:], ones_f[:])
            gs = cpool.tile([P, 130], F32)
            with (
                tc.tile_pool(name="gacc", bufs=1, space="PSUM") as gpool,
                tc.tile_pool(name="tps", bufs=4, space="PSUM") as tpool,
            ):
                g_ps = gpool.tile([P, 130], F32)
                for j in range(N_GCH):
                    sl = slice(j * GCHUNK, (j + 1) * GCHUNK)
                    tps = tpool.tile([128, GCHUNK], F32)
                    nc.tensor.transpose(tps[:], xs[:, sl], ident[:])
                    xt = xtpool.tile([128, 130], F32R)
                    nc.gpsimd.tensor_copy(xt[:, 128:130], ones_r[:])
                    if j % 2 == 0:
                        nc.vector.tensor_copy(xt[:, 0:128], tps[:])
                    else:
                        nc.scalar.copy(xt[:, 0:128], tps[:])
                    nc.tensor.matmul(
                        g_ps[:],
                        lhsT=xt[:, 0:128],
                        rhs=xt[:, 0:130],
                        start=(j == 0),
                        stop=(j == N_GCH - 1),
                    )

                nc.vector.tensor_copy(gs[:], g_ps[:])

            # ---- all-reduce [G|s] over the 8 cores (HBM bounce) ----
            cc_in = dram.tile([P, 130], F32)
            cc_out = dram.tile([P, 130], F32)
            nc.sync.dma_start(cc_in, gs[:])
            nc.gpsimd.collective_compute(
                "AllReduce",
                mybir.AluOpType.add,
                replica_groups=[list(range(NCORES))],
                ins=[cc_in.opt()],
                outs=[cc_out.opt()],
            )
            gsr = cpool.tile([P, 130], F32)
            nc.sync.dma_start(gsr[:], cc_out)

            # ---- tiny math: S = wqa^T G_aug wka ; softmax ; QT, c ----
            mpool = ctx_mpool = tc.alloc_tile_pool(name="pmath", bufs=1, space="PSUM")
            # s^T row via PE transpose of the s column
            st_ps = mpool.tile([1, 128], F32, tag="m1")
            nc.tensor.transpose(st_ps[:], gsr[:, 128:129], ident[:])
            st = cpool.tile([1, 128], F32)
            nc.vector.tensor_copy(st[:], st_ps[:])

            ga = []
            for b in range(B):
                g_aug = cpool.tile([65, 65], F32, tag=f"ga{b}", name=f"g_aug{b}")
                cs = slice(b * 64, (b + 1) * 64)
                nc.vector.tensor_copy(g_aug[0:64, 0:64], gsr[cs, cs])
                nc.vector.tensor_copy(g_aug[0:64, 64:65], gsr[cs, 128:129])
                nc.vector.tensor_copy(g_aug[64:65, 0:64], st[:, cs])
                nc.vector.memset(g_aug[64:65, 64:65], float(N_TOTAL))
                ga.append(g_aug)

            # A_b = G_aug_b @ wka  (G_aug symmetric -> lhsT = G_aug)
            s_ps = mpool.tile([P, 64], F32, tag="m2")
            for b in range(B):
                a_ps = mpool.tile([65, 64], F32, tag="m1", name=f"a_ps{b}")
                nc.tensor.matmul(a_ps[:], lhsT=ga[b][:], rhs=wka[:])
                a_sb = cpool.tile([65, 64], F32, tag=f"asb{b}", name=f"a_sb{b}")
                nc.vector.tensor_copy(a_sb[:], a_ps[:])
                # S_b = wqa^T @ A_b   (scale 1/8 folded into wqa)
                nc.tensor.matmul(
                    s_ps[b * 64 : (b + 1) * 64, :], lhsT=wqa[:], rhs=a_sb[:]
                )

            # softmax rows (both batches stacked [128, 64])
            negmax = cpool.tile([P, 1], F32)
            nc.vector.reduce_max(negmax[:], s_ps[:], axis=mybir.AxisListType.X,
                                 negate=True)
            expv = cpool.tile([P, 64], F32)
            rowsum = cpool.tile([P, 1], F32)
            nc.scalar.activation(
                expv[:], s_ps[:], mybir.ActivationFunctionType.Exp,
                bias=negmax[:, 0:1], scale=1.0, accum_out=rowsum[:, 0:1],
            )
            rinv = cpool.tile([P, 1], F32)
            nc.vector.reciprocal(rinv[:], rowsum[:])
            attn = cpool.tile([P, 64], F32)
            nc.vector.tensor_scalar_mul(attn[:], expv[:], rinv[:, 0:1])

            # attn^T (one transpose: [128,64] -> [64,128] = [attn0^T | attn1^T])
            at_ps = mpool.tile([64, 128], F32, tag="m1")
            nc.tensor.transpose(at_ps[:], attn[:], ident[:])
            at_sb = cpool.tile([64, 128], F32)
            nc.vector.tensor_copy(at_sb[:], at_ps[:])

            # QT block-diag [128,128]: QT_b = wv^T attn_b^T + I
            qt_ps = mpool.tile([128, 128], F32, tag="m2")
            c_ps = mpool.tile([128, 1], F32, tag="m3")
            for b in range(B):
                cs = slice(b * 64, (b + 1) * 64)
                nc.tensor.matmul(
                    qt_ps[cs, cs], lhsT=wv[:], rhs=at_sb[:, cs],
                    start=True, stop=False,
                )
                nc.tensor.matmul(
                    qt_ps[cs, cs], lhsT=ident[0:64, 0:64], rhs=ident[0:64, 0:64],
                    start=False, stop=True,
                )
                nc.tensor.matmul(c_ps[cs, :], lhsT=at_sb[:, cs], rhs=bv[:])
            qt = cpool.tile([128, 128], F32)
            nc.vector.memset(qt[:], 0.0)
            for b in range(B):
                cs = slice(b * 64, (b + 1) * 64)
                nc.vector.tensor_copy(qt[cs, cs], qt_ps[cs, cs])
            cvec = cpool.tile([P, 1], F32)
            nc.vector.tensor_copy(cvec[:], c_ps[:])
            ctx_mpool.release()

            # ---- phase 2: out = QT^T x + c ----
            with tc.tile_pool(name="ops", bufs=4, space="PSUM") as oppool:
                for k in range(N_OCH):
                    sl = slice(k * OCHUNK, (k + 1) * OCHUNK)
                    o_ps = oppool.tile([P, OCHUNK], F32)
                    nc.tensor.matmul(o_ps[:], lhsT=qt[:], rhs=xs[:, sl])
                    osb = opool.tile([P, OCHUNK], F32)
                    if k % 2 == 0:
                        nc.vector.tensor_scalar_add(osb[:], o_ps[:], cvec[:, 0:1])
                    else:
                        nc.scalar.add(osb[:], o_ps[:], cvec[:, 0:1])
                    nc.sync.dma_start(out_t[:, sl], osb[:])

    nc.compile()
    return nc


_cached_nc = None


def kernel(x, wq, bq, wk, bk, wv, bv, _trace=False):
    global _cached_nc
    x = np.ascontiguousarray(np.asarray(x, dtype=np.float32))
    assert x.shape == (B, C, 64, 64, 64)
    xf = x.reshape(P, N_TOTAL)

    wqa = (
        np.concatenate(
            [np.asarray(wq, np.float64), np.asarray(bq, np.float64)[:, None]], axis=1
        ).T
        / 8.0
    ).astype(np.float32)  # [65, 64]
    wka = (
        np.concatenate(
            [np.asarray(wk, np.float64), np.asarray(bk, np.float64)[:, None]], axis=1
        ).T
    ).astype(np.float32)  # [65, 64]
    wv32 = np.ascontiguousarray(np.asarray(wv, np.float32))
    bv32 = np.ascontiguousarray(np.asarray(bv, np.float32).reshape(64, 1))
    ident = np.eye(128, dtype=np.float32)

    in_maps = []
    for i in range(NCORES):
        sl = slice(i * N_SHARD, (i + 1) * N_SHARD)
        in_maps.append(
            {
                "x": np.ascontiguousarray(xf[:, sl]),
                "wqa": wqa,
                "wka": wka,
                "wv": wv32,
                "bv": bv32,
                "ident": ident,
            }
        )

    if _cached_nc is None:
        _cached_nc = build_bass()
    nc = _cached_nc

    res = bass_utils.run_bass_kernel_spmd(
        nc, in_maps, core_ids=list(range(NCORES)), trace=_trace
    )
    kernel._last_results = res

    out = np.empty((P, N_TOTAL), dtype=np.float32)
    for i in range(NCORES):
        out[:, i * N_SHARD : (i + 1) * N_SHARD] = res.results[i]["out"]
    return out.reshape(B, C, 64, 64, 64)


kernel._last_results = None


# revision 11
# speedup vs baseline: 1.7214x; 1.3175x over previous
"""Channel-attention block (AttentionBlock, C=64) on 8 trn2 NeuronCores.

Algebraic reduction: with q = wq x + bq etc. and attention over channels,
    S  = q k^T / sqrt(C) = wqa^T_aug G_aug wka_aug / 8,   G_aug = [[x x^T, s],[s^T, N]]
    out = softmax(S) v + x = (attn wv + I) x + (attn bv) 1^T
so the kernel only needs the 65x65 Gram (per batch) of x plus one matmul pass
over x.  The N axis is sharded over 8 cores; the [G|s] partial sums (66 KB)
are all-reduced on-device.

Layout: batches stacked on partitions (p = b*64 + c) so matmuls run K=M=128
with block-diagonal weights.  The Gram contraction needs n on partitions, so
the host supplies an fp16 copy of x pre-permuted to [p, q, c] (q indexes
128-position chunks); the Gram is then 2 fp16 matmuls per chunk with zero
on-device transposes.  Phase 2 runs fp32r (single-pass) with the fp32
residual exact.
"""

import numpy as np

import concourse.bacc as bacc
import concourse.mybir as mybir
import concourse.tile as tile
from concourse import bass_utils

F32 = mybir.dt.float32
F32R = mybir.dt.float32r
F16 = mybir.dt.float16

NCORES = 8
B, C = 2, 64
P = B * C  # 128 partitions, batches stacked
N_TOTAL = 64 * 64 * 64  # 262144
N_SHARD = N_TOTAL // NCORES  # 32768
GCHUNK = 128
N_GCH = N_SHARD // GCHUNK  # 256
SLAB = 32  # gram chunks per fp16 slab load
N_SLAB = N_GCH // SLAB  # 8
OCHUNK = 512  # phase-2 matmul free dim
OSTORE = 1024  # output store width
LDCHUNK = 2048  # fp32 input DMA slice
N_LDCH = N_SHARD // LDCHUNK  # 16


def build_bass():
    nc = bacc.Bacc(
        "TRN2",
        target_bir_lowering=False,
        debug=False,
        num_devices=NCORES,
    )

    x_t = nc.dram_tensor("x", [P, N_SHARD], F32R, kind="ExternalInput")
    xh_t = nc.dram_tensor("xh", [P, N_GCH, GCHUNK], F16, kind="ExternalInput")
    wqa_t = nc.dram_tensor("wqa", [65, 64], F32, kind="ExternalInput")  # [wq|bq]^T/8
    wka_t = nc.dram_tensor("wka", [65, 64], F32, kind="ExternalInput")  # [wk|bk]^T
    wv_t = nc.dram_tensor("wv", [64, 64], F32, kind="ExternalInput")
    bv_t = nc.dram_tensor("bv", [64, 1], F32, kind="ExternalInput")
    id_t = nc.dram_tensor("ident", [128, 128], F32, kind="ExternalInput")
    out_t = nc.dram_tensor("out", [P, N_SHARD], F32, kind="ExternalOutput")

    with tile.TileContext(nc, num_cores=NCORES) as tc:
        with (
            tc.tile_pool(name="xbuf", bufs=1) as xpool,
            tc.tile_pool(name="consts", bufs=1) as cpool,
            tc.tile_pool(name="slab", bufs=3) as spool,
            tc.tile_pool(name="osb", bufs=4) as opool,
            tc.tile_pool(name="dram", bufs=2, space="DRAM") as dram,
        ):
            # ---- constants to SBUF ----
            ident = cpool.tile([128, 128], F32)
            nc.scalar.dma_start(ident[:], id_t[:, :])
            wqa = cpool.tile([65, 64], F32)
            nc.scalar.dma_start(wqa[:], wqa_t[:, :])
            wka = cpool.tile([65, 64], F32)
            nc.scalar.dma_start(wka[:], wka_t[:, :])
            wv = cpool.tile([64, 64], F32)
            nc.scalar.dma_start(wv[:], wv_t[:, :])
            bv = cpool.tile([64, 1], F32)
            nc.scalar.dma_start(bv[:], bv_t[:, :])
            ones_h = cpool.tile([128, 2], F16)
            nc.vector.memset(ones_h[:], 1.0)

            # ---- phase 1: G_psum[:,0:128] += xT^T xT ; cols 128:130 = sums ----
            gs = cpool.tile([P, 130], F32)
            with tc.tile_pool(name="gacc", bufs=1, space="PSUM") as gpool:
                g_ps = gpool.tile([P, 130], F32)
                for t in range(N_SLAB):
                    slab = spool.tile([P, SLAB, GCHUNK], F16)
                    nc.scalar.dma_start(slab[:], xh_t[:, t * SLAB : (t + 1) * SLAB, :])
                    for q in range(SLAB):
                        j = t * SLAB + q
                        nc.tensor.matmul(
                            g_ps[:, 0:128],
                            lhsT=slab[:, q, :],
                            rhs=slab[:, q, :],
                            start=(j == 0),
                            stop=(j == N_GCH - 1),
                        )
                        nc.tensor.matmul(
                            g_ps[:, 128:130],
                            lhsT=slab[:, q, :],
                            rhs=ones_h[:],
                            start=(j == 0),
                            stop=(j == N_GCH - 1),
                        )
                nc.vector.tensor_copy(gs[:], g_ps[:])

            # ---- fp32 x shard load (only consumed by phase 2; streams through
            # phase 1 + the collective bubble on the sync queues) ----
            xs = xpool.tile([P, N_SHARD], F32R)
            for k in range(N_LDCH):
                sl = slice(k * LDCHUNK, (k + 1) * LDCHUNK)
                nc.sync.dma_start(xs[:, sl], x_t[:, sl])

            # ---- all-reduce [G|s] over the 8 cores (HBM bounce) ----
            cc_in = dram.tile([P, 130], F32)
            cc_out = dram.tile([P, 130], F32)
            nc.scalar.dma_start(cc_in, gs[:])
            nc.gpsimd.collective_compute(
                "AllReduce",
                mybir.AluOpType.add,
                replica_groups=[list(range(NCORES))],
                ins=[cc_in.opt()],
                outs=[cc_out.opt()],
            )
            gsr = cpool.tile([P, 130], F32)
            nc.scalar.dma_start(gsr[:], cc_out)

            # ---- tiny math: S = wqa^T G_aug wka ; softmax ; QT, c ----
            mpool = tc.alloc_tile_pool(name="pmath", bufs=1, space="PSUM")
            # s^T row via PE transpose of the s column
            st_ps = mpool.tile([1, 128], F32, tag="m1")
            nc.tensor.transpose(st_ps[:], gsr[:, 128:129], ident[:])
            st = cpool.tile([1, 128], F32)
            nc.vector.tensor_copy(st[:], st_ps[:])

            ga = []
            for b in range(B):
                g_aug = cpool.tile([65, 65], F32, tag=f"ga{b}", name=f"g_aug{b}")
                cs = slice(b * 64, (b + 1) * 64)
                nc.vector.tensor_copy(g_aug[0:64, 0:64], gsr[cs, cs])
                nc.vector.tensor_copy(g_aug[0:64, 64:65], gsr[cs, 128:129])
                nc.vector.tensor_copy(g_aug[64:65, 0:64], st[:, cs])
                nc.vector.memset(g_aug[64:65, 64:65], float(N_TOTAL))
                ga.append(g_aug)

            # A_b = G_aug_b @ wka  (G_aug symmetric -> lhsT = G_aug)
            s_ps = mpool.tile([P, 64], F32, tag="m2")
            for b in range(B):
                a_ps = mpool.tile([65, 64], F32, tag="m1", name=f"a_ps{b}")
                nc.tensor.matmul(a_ps[:], lhsT=ga[b][:], rhs=wka[:])
                a_sb = cpool.tile([65, 64], F32, tag=f"asb{b}", name=f"a_sb{b}")
                nc.vector.tensor_copy(a_sb[:], a_ps[:])
                # S_b = wqa^T @ A_b   (1/8 scale folded into wqa)
                nc.tensor.matmul(
                    s_ps[b * 64 : (b + 1) * 64, :], lhsT=wqa[:], rhs=a_sb[:]
                )

            # softmax rows (both batches stacked [128, 64])
            negmax = cpool.tile([P, 1], F32)
            nc.vector.reduce_max(
                negmax[:], s_ps[:], axis=mybir.AxisListType.X, negate=True
            )
            expv = cpool.tile([P, 64], F32)
            rowsum = cpool.tile([P, 1], F32)
            nc.scalar.activation(
                expv[:], s_ps[:], mybir.ActivationFunctionType.Exp,
                bias=negmax[:, 0:1], scale=1.0, accum_out=rowsum[:, 0:1],
            )
            rinv = cpool.tile([P, 1], F32)
            nc.vector.reciprocal(rinv[:], rowsum[:])
            attn = cpool.tile([P, 64], F32)
            nc.vector.tensor_scalar_mul(attn[:], expv[:], rinv[:, 0:1])

            # attn^T (one transpose: [128,64] -> [64,128] = [attn0^T | attn1^T])
            at_ps = mpool.tile([64, 128], F32, tag="m1")
            nc.tensor.transpose(at_ps[:], attn[:], ident[:])
            at_sb = cpool.tile([64, 128], F32)
            nc.vector.tensor_copy(at_sb[:], at_ps[:])

            # QT block-diag [128,128]: QT_b = wv^T attn_b^T + I
            qt_ps = mpool.tile([128, 128], F32, tag="m2")
            c_ps = mpool.tile([128, 1], F32, tag="m3")
            for b in range(B):
                cs = slice(b * 64, (b + 1) * 64)
                nc.tensor.matmul(
                    qt_ps[cs, cs], lhsT=wv[:], rhs=at_sb[:, cs],
                    start=True, stop=False,
                )
                nc.tensor.matmul(
                    qt_ps[cs, cs], lhsT=ident[0:64, 0:64], rhs=ident[0:64, 0:64],
                    start=False, stop=True,
                )
                nc.tensor.matmul(c_ps[cs, :], lhsT=at_sb[:, cs], rhs=bv[:])
            qt = cpool.tile([128, 128], F32)
            nc.vector.memset(qt[:], 0.0)
            for b in range(B):
                cs = slice(b * 64, (b + 1) * 64)
                nc.vector.tensor_copy(qt[cs, cs], qt_ps[cs, cs])
            qt_r = cpool.tile([128, 128], F32R)
            nc.vector.tensor_copy(qt_r[:], qt[:])
            cvec = cpool.tile([P, 1], F32)
            nc.vector.tensor_copy(cvec[:], c_ps[:])
            mpool.release()

            # ---- phase 2: out = QT^T x + c  (fp32r single-pass matmuls) ----
            with tc.tile_pool(name="ops", bufs=6, space="PSUM") as oppool:
                for k in range(N_SHARD // OSTORE):
                    osb = opool.tile([P, OSTORE], F32)
                    for h in range(2):
                        sl = slice(k * OSTORE + h * OCHUNK, k * OSTORE + (h + 1) * OCHUNK)
                        o_ps = oppool.tile([P, OCHUNK], F32)
                        nc.tensor.matmul(o_ps[:], lhsT=qt_r[:], rhs=xs[:, sl])
                        oslice = osb[:, h * OCHUNK : (h + 1) * OCHUNK]
                        if h == 0:
                            nc.vector.tensor_scalar_add(oslice, o_ps[:], cvec[:, 0:1])
                        else:
                            nc.scalar.add(oslice, o_ps[:], cvec[:, 0:1])
                    nc.sync.dma_start(
                        out_t[:, k * OSTORE : (k + 1) * OSTORE], osb[:]
                    )

    nc.compile()
    return nc


_cached_nc = None


def kernel(x, wq, bq, wk, bk, wv, bv, _trace=False):
    global _cached_nc
    x = np.ascontiguousarray(np.asarray(x, dtype=np.float32))
    assert x.shape == (B, C, 64, 64, 64)
    xf = x.reshape(P, N_TOTAL)

    wqa = (
        np.concatenate(
            [np.asarray(wq, np.float64), np.asarray(bq, np.float64)[:, None]], axis=1
        ).T
        / 8.0
    ).astype(np.float32)  # [65, 64]
    wka = (
        np.concatenate(
            [np.asarray(wk, np.float64), np.asarray(bk, np.float64)[:, None]], axis=1
        ).T
    ).astype(np.float32)  # [65, 64]
    wv32 = np.ascontiguousarray(np.asarray(wv, np.float32))
    bv32 = np.ascontiguousarray(np.asarray(bv, np.float32).reshape(64, 1))
    ident = np.eye(128, dtype=np.float32)

    in_maps = []
    for i in range(NCORES):
        sl = slice(i * N_SHARD, (i + 1) * N_SHARD)
        xsh = np.ascontiguousarray(xf[:, sl])
        # xh[p, q, c] = x[c, q*128 + p] in fp16 (gram operand, n on partitions)
        xh = np.ascontiguousarray(
            xsh.astype(np.float16).reshape(P, N_GCH, GCHUNK).transpose(2, 1, 0)
        )
        in_maps.append(
            {
                "x": xsh,
                "xh": xh,
                "wqa": wqa,
                "wka": wka,
                "wv": wv32,
                "bv": bv32,
                "ident": ident,
            }
        )

    if _cached_nc is None:
        _cached_nc = build_bass()
    nc = _cached_nc

    res = bass_utils.run_bass_kernel_spmd(
        nc, in_maps, core_ids=list(range(NCORES)), trace=_trace
    )
    kernel._last_results = res

    out = np.empty((P, N_TOTAL), dtype=np.float32)
    for i in range(NCORES):
        out[:, i * N_SHARD : (i + 1) * N_SHARD] = res.results[i]["out"]
    return out.reshape(B, C, 64, 64, 64)


kernel._last_results = None


# revision 12
# speedup vs baseline: 1.7243x; 1.0017x over previous
"""Channel-attention block (AttentionBlock, C=64) on 8 trn2 NeuronCores.

Algebraic reduction: with q = wq x + bq etc. and attention over channels,
    S  = q k^T / sqrt(C) = wqa^T_aug G_aug wka_aug / 8,   G_aug = [[x x^T, s],[s^T, N]]
    out = softmax(S) v + x = (attn wv + I) x + (attn bv) 1^T
so the kernel only needs the 65x65 Gram (per batch) of x plus one matmul pass
over x.  The N axis is sharded over 8 cores; the [G|s] partial sums (66 KB)
are all-reduced on-device.

Layout: batches stacked on partitions (p = b*64 + c) so matmuls run K=M=128
with block-diagonal weights.  The Gram contraction needs n on partitions, so
the host supplies an fp16 copy of x pre-permuted to [p, q, c] (q indexes
128-position chunks); the Gram is then 2 fp16 matmuls per chunk with zero
on-device transposes.  Phase 2 runs fp32r (single-pass) with the fp32
residual exact.
"""

import ml_dtypes
import numpy as np

import concourse.bacc as bacc
import concourse.mybir as mybir
import concourse.tile as tile
from concourse import bass_utils

F32 = mybir.dt.float32
F32R = mybir.dt.float32r
BF16 = mybir.dt.bfloat16

NCORES = 8
B, C = 2, 64
P = B * C  # 128 partitions, batches stacked
N_TOTAL = 64 * 64 * 64  # 262144
N_SHARD = N_TOTAL // NCORES  # 32768
GCHUNK = 128
N_GCH = N_SHARD // GCHUNK  # 256
SLAB = 32  # gram chunks per fp16 slab load
N_SLAB = N_GCH // SLAB  # 8
OCHUNK = 512  # phase-2 matmul free dim
OSTORE = 1024  # output store width
LDCHUNK = 2048  # fp32 input DMA slice
N_LDCH = N_SHARD // LDCHUNK  # 16


def build_bass():
    nc = bacc.Bacc(
        "TRN2",
        target_bir_lowering=False,
        debug=False,
        num_devices=NCORES,
    )

    x_t = nc.dram_tensor("x", [P, N_SHARD], F32R, kind="ExternalInput")
    xh_t = nc.dram_tensor("xh", [P, N_GCH, GCHUNK], BF16, kind="ExternalInput")
    wqa_t = nc.dram_tensor("wqa", [65, 64], F32, kind="ExternalInput")  # [wq|bq]^T/8
    wka_t = nc.dram_tensor("wka", [65, 64], F32, kind="ExternalInput")  # [wk|bk]^T
    wv_t = nc.dram_tensor("wv", [64, 64], F32, kind="ExternalInput")
    bv_t = nc.dram_tensor("bv", [64, 1], F32, kind="ExternalInput")
    id_t = nc.dram_tensor("ident", [128, 128], F32, kind="ExternalInput")
    out_t = nc.dram_tensor("out", [P, N_SHARD], F32, kind="ExternalOutput")

    with tile.TileContext(nc, num_cores=NCORES) as tc:
        with (
            tc.tile_pool(name="xbuf", bufs=1) as xpool,
            tc.tile_pool(name="consts", bufs=1) as cpool,
            tc.tile_pool(name="slab", bufs=3) as spool,
            tc.tile_pool(name="osb", bufs=4) as opool,
            tc.tile_pool(name="dram", bufs=2, space="DRAM") as dram,
        ):
            # ---- constants to SBUF ----
            ident = cpool.tile([128, 128], F32)
            nc.scalar.dma_start(ident[:], id_t[:, :])
            wqa = cpool.tile([65, 64], F32)
            nc.scalar.dma_start(wqa[:], wqa_t[:, :])
            wka = cpool.tile([65, 64], F32)
            nc.scalar.dma_start(wka[:], wka_t[:, :])
            wv = cpool.tile([64, 64], F32)
            nc.scalar.dma_start(wv[:], wv_t[:, :])
            bv = cpool.tile([64, 1], F32)
            nc.scalar.dma_start(bv[:], bv_t[:, :])
            ones_h = cpool.tile([128, 2], BF16)
            nc.vector.memset(ones_h[:], 1.0)

            # ---- fp32 x shard load (only consumed by phase 2; streams through
            # phase 1 + the collective bubble on the sync queues) ----
            xs = xpool.tile([P, N_SHARD], F32R)
            for k in range(N_LDCH):
                sl = slice(k * LDCHUNK, (k + 1) * LDCHUNK)
                nc.sync.dma_start(xs[:, sl], x_t[:, sl])

            # ---- phase 1: G_psum[:,0:128] += xT^T xT ; cols 128:130 = sums ----
            gs = cpool.tile([P, 130], F32)
            with tc.tile_pool(name="gacc", bufs=1, space="PSUM") as gpool:
                g_ps = gpool.tile([P, 130], F32)
                for t in range(N_SLAB):
                    slab = spool.tile([P, SLAB, GCHUNK], BF16)
                    nc.scalar.dma_start(slab[:], xh_t[:, t * SLAB : (t + 1) * SLAB, :])
                    for q in range(SLAB):
                        j = t * SLAB + q
                        nc.tensor.matmul(
                            g_ps[:, 0:128],
                            lhsT=slab[:, q, :],
                            rhs=slab[:, q, :],
                            start=(j == 0),
                            stop=(j == N_GCH - 1),
                        )
                        nc.tensor.matmul(
                            g_ps[:, 128:130],
                            lhsT=slab[:, q, :],
                            rhs=ones_h[:],
                            start=(j == 0),
                            stop=(j == N_GCH - 1),
                        )
                nc.vector.tensor_copy(gs[:], g_ps[:])

            # ---- all-reduce [G|s] over the 8 cores (HBM bounce) ----
            cc_in = dram.tile([P, 130], F32)
            cc_out = dram.tile([P, 130], F32)
            nc.scalar.dma_start(cc_in, gs[:])
            nc.gpsimd.collective_compute(
                "AllReduce",
                mybir.AluOpType.add,
                replica_groups=[list(range(NCORES))],
                ins=[cc_in.opt()],
                outs=[cc_out.opt()],
            )
            gsr = cpool.tile([P, 130], F32)
            nc.scalar.dma_start(gsr[:], cc_out)

            # ---- tiny math: S = wqa^T G_aug wka ; softmax ; QT, c ----
            mpool = tc.alloc_tile_pool(name="pmath", bufs=1, space="PSUM")
            # s^T row via PE transpose of the s column
            st_ps = mpool.tile([1, 128], F32, tag="m1")
            nc.tensor.transpose(st_ps[:], gsr[:, 128:129], ident[:])
            st = cpool.tile([1, 128], F32)
            nc.vector.tensor_copy(st[:], st_ps[:])

            ga = []
            for b in range(B):
                g_aug = cpool.tile([65, 65], F32, tag=f"ga{b}", name=f"g_aug{b}")
                cs = slice(b * 64, (b + 1) * 64)
                nc.vector.tensor_copy(g_aug[0:64, 0:64], gsr[cs, cs])
                nc.vector.tensor_copy(g_aug[0:64, 64:65], gsr[cs, 128:129])
                nc.vector.tensor_copy(g_aug[64:65, 0:64], st[:, cs])
                nc.vector.memset(g_aug[64:65, 64:65], float(N_TOTAL))
                ga.append(g_aug)

            # A_b = G_aug_b @ wka  (G_aug symmetric -> lhsT = G_aug)
            s_ps = mpool.tile([P, 64], F32, tag="m2")
            for b in range(B):
                a_ps = mpool.tile([65, 64], F32, tag="m1", name=f"a_ps{b}")
                nc.tensor.matmul(a_ps[:], lhsT=ga[b][:], rhs=wka[:])
                a_sb = cpool.tile([65, 64], F32, tag=f"asb{b}", name=f"a_sb{b}")
                nc.vector.tensor_copy(a_sb[:], a_ps[:])
                # S_b = wqa^T @ A_b   (1/8 scale folded into wqa)
                nc.tensor.matmul(
                    s_ps[b * 64 : (b + 1) * 64, :], lhsT=wqa[:], rhs=a_sb[:]
                )

            # softmax rows (both batches stacked [128, 64])
            negmax = cpool.tile([P, 1], F32)
            nc.vector.reduce_max(
                negmax[:], s_ps[:], axis=mybir.AxisListType.X, negate=True
            )
            expv = cpool.tile([P, 64], F32)
            rowsum = cpool.tile([P, 1], F32)
            nc.scalar.activation(
                expv[:], s_ps[:], mybir.ActivationFunctionType.Exp,
                bias=negmax[:, 0:1], scale=1.0, accum_out=rowsum[:, 0:1],
            )
            rinv = cpool.tile([P, 1], F32)
            nc.vector.reciprocal(rinv[:], rowsum[:])
            attn = cpool.tile([P, 64], F32)
            nc.vector.tensor_scalar_mul(attn[:], expv[:], rinv[:, 0:1])

            # attn^T (one transpose: [128,64] -> [64,128] = [attn0^T | attn1^T])
            at_ps = mpool.tile([64, 128], F32, tag="m1")
            nc.tensor.transpose(at_ps[:], attn[:], ident[:])
            at_sb = cpool.tile([64, 128], F32)
            nc.vector.tensor_copy(at_sb[:], at_ps[:])

            # QT block-diag [128,128]: QT_b = wv^T attn_b^T + I
            qt_ps = mpool.tile([128, 128], F32, tag="m2")
            c_ps = mpool.tile([128, 1], F32, tag="m3")
            for b in range(B):
                cs = slice(b * 64, (b + 1) * 64)
                nc.tensor.matmul(
                    qt_ps[cs, cs], lhsT=wv[:], rhs=at_sb[:, cs],
                    start=True, stop=False,
                )
                nc.tensor.matmul(
                    qt_ps[cs, cs], lhsT=ident[0:64, 0:64], rhs=ident[0:64, 0:64],
                    start=False, stop=True,
                )
                nc.tensor.matmul(c_ps[cs, :], lhsT=at_sb[:, cs], rhs=bv[:])
            qt = cpool.tile([128, 128], F32)
            nc.vector.memset(qt[:], 0.0)
            for b in range(B):
                cs = slice(b * 64, (b + 1) * 64)
                nc.vector.tensor_copy(qt[cs, cs], qt_ps[cs, cs])
            qt_r = cpool.tile([128, 128], F32R)
            nc.vector.tensor_copy(qt_r[:], qt[:])
            cvec = cpool.tile([P, 1], F32)
            nc.vector.tensor_copy(cvec[:], c_ps[:])
            mpool.release()

            # ---- phase 2: out = QT^T x + c  (fp32r single-pass matmuls) ----
            with tc.tile_pool(name="ops", bufs=6, space="PSUM") as oppool:
                for k in range(N_SHARD // OSTORE):
                    osb = opool.tile([P, OSTORE], F32)
                    for h in range(2):
                        sl = slice(k * OSTORE + h * OCHUNK, k * OSTORE + (h + 1) * OCHUNK)
                        o_ps = oppool.tile([P, OCHUNK], F32)
                        nc.tensor.matmul(o_ps[:], lhsT=qt_r[:], rhs=xs[:, sl])
                        oslice = osb[:, h * OCHUNK : (h + 1) * OCHUNK]
                        if h == 0:
                            nc.vector.tensor_scalar_add(oslice, o_ps[:], cvec[:, 0:1])
                        else:
                            nc.scalar.add(oslice, o_ps[:], cvec[:, 0:1])
                    nc.sync.dma_start(
                        out_t[:, k * OSTORE : (k + 1) * OSTORE], osb[:]
                    )

    nc.compile()
    return nc


_cached_nc = None


def kernel(x, wq, bq, wk, bk, wv, bv, _trace=False):
    global _cached_nc
    x = np.ascontiguousarray(np.asarray(x, dtype=np.float32))
    assert x.shape == (B, C, 64, 64, 64)
    xf = x.reshape(P, N_TOTAL)

    wqa = (
        np.concatenate(
            [np.asarray(wq, np.float64), np.asarray(bq, np.float64)[:, None]], axis=1
        ).T
        / 8.0
    ).astype(np.float32)  # [65, 64]
    wka = (
        np.concatenate(
            [np.asarray(wk, np.float64), np.asarray(bk, np.float64)[:, None]], axis=1
        ).T
    ).astype(np.float32)  # [65, 64]
    wv32 = np.ascontiguousarray(np.asarray(wv, np.float32))
    bv32 = np.ascontiguousarray(np.asarray(bv, np.float32).reshape(64, 1))
    ident = np.eye(128, dtype=np.float32)

    in_maps = []
    for i in range(NCORES):
        sl = slice(i * N_SHARD, (i + 1) * N_SHARD)
        xsh = np.ascontiguousarray(xf[:, sl])
        # xh[p, q, c] = x[c, q*128 + p] in fp16 (gram operand, n on partitions)
        xh = np.ascontiguousarray(
            xsh.astype(ml_dtypes.bfloat16).reshape(P, N_GCH, GCHUNK).transpose(2, 1, 0)
        )
        in_maps.append(
            {
                "x": xsh,
                "xh": xh,
                "wqa": wqa,
                "wka": wka,
                "wv": wv32,
                "bv": bv32,
                "ident": ident,
            }
        )

    if _cached_nc is None:
        _cached_nc = build_bass()
    nc = _cached_nc

    res = bass_utils.run_bass_kernel_spmd(
        nc, in_maps, core_ids=list(range(NCORES)), trace=_trace
    )
    kernel._last_results = res

    out = np.empty((P, N_TOTAL), dtype=np.float32)
    for i in range(NCORES):
        out[:, i * N_SHARD : (i + 1) * N_SHARD] = res.results[i]["out"]
    return out.reshape(B, C, 64, 64, 64)


kernel._last_results = None


# revision 13
# speedup vs baseline: 1.8554x; 1.0761x over previous
"""Channel-attention block (AttentionBlock, C=64) on 8 trn2 NeuronCores.

Algebraic reduction: with q = wq x + bq etc. and attention over channels,
    S  = q k^T / sqrt(C) = wqa^T_aug G_aug wka_aug / 8,   G_aug = [[x x^T, s],[s^T, N]]
    out = softmax(S) v + x = (attn wv + I) x + (attn bv) 1^T
so the kernel only needs the 65x65 Gram (per batch) of x plus one matmul pass
over x.  The N axis is sharded over 8 cores; the [G|s] partial sums (66 KB)
are all-reduced on-device.

Layout: batches stacked on partitions (p = b*64 + c) so matmuls run K=M=128
with block-diagonal weights.  The Gram contraction needs n on partitions, so
the host supplies an fp16 copy of x pre-permuted to [p, q, c] (q indexes
128-position chunks); the Gram is then 2 fp16 matmuls per chunk with zero
on-device transposes.  Phase 2 runs fp32r (single-pass) with the fp32
residual exact.
"""

import ml_dtypes
import numpy as np

import concourse.bacc as bacc
import concourse.mybir as mybir
import concourse.tile as tile
from concourse import bass_utils

F32 = mybir.dt.float32
F32R = mybir.dt.float32r
BF16 = mybir.dt.bfloat16

NCORES = 8
B, C = 2, 64
P = B * C  # 128 partitions, batches stacked
N_TOTAL = 64 * 64 * 64  # 262144
N_SHARD = N_TOTAL // NCORES  # 32768
GCHUNK = 128
N_GCH = N_SHARD // GCHUNK  # 256
SLAB = 32  # gram chunks per fp16 slab load
N_SLAB = N_GCH // SLAB  # 8
OCHUNK = 512  # phase-2 matmul free dim
OSTORE = 1024  # output store width
LDCHUNK = 2048  # fp32 input DMA slice
N_LDCH = N_SHARD // LDCHUNK  # 16


def build_bass():
    nc = bacc.Bacc(
        "TRN2",
        target_bir_lowering=False,
        debug=False,
        num_devices=NCORES,
    )

    x_t = nc.dram_tensor("x", [P, N_SHARD], F32R, kind="ExternalInput")
    xh_t = nc.dram_tensor("xh", [P, N_GCH, GCHUNK], BF16, kind="ExternalInput")
    wqa_t = nc.dram_tensor("wqa", [65, 64], F32, kind="ExternalInput")  # [wq|bq]^T/8
    wka_t = nc.dram_tensor("wka", [65, 64], F32, kind="ExternalInput")  # [wk|bk]^T
    wv_t = nc.dram_tensor("wv", [64, 64], F32, kind="ExternalInput")
    bv_t = nc.dram_tensor("bv", [64, 1], F32, kind="ExternalInput")
    id_t = nc.dram_tensor("ident", [128, 128], F32, kind="ExternalInput")
    out_t = nc.dram_tensor("out", [P, N_SHARD], F32, kind="ExternalOutput")

    with tile.TileContext(nc, num_cores=NCORES) as tc:
        with (
            tc.tile_pool(name="xbuf", bufs=1) as xpool,
            tc.tile_pool(name="consts", bufs=1) as cpool,
            tc.tile_pool(name="slab", bufs=3) as spool,
            tc.tile_pool(name="osb", bufs=4) as opool,
            tc.tile_pool(name="dram", bufs=2, space="DRAM") as dram,
        ):
            # ---- constants to SBUF ----
            ident = cpool.tile([128, 128], F32)
            nc.scalar.dma_start(ident[:], id_t[:, :])
            wqa = cpool.tile([65, 64], F32)
            nc.scalar.dma_start(wqa[:], wqa_t[:, :])
            wka = cpool.tile([65, 64], F32)
            nc.scalar.dma_start(wka[:], wka_t[:, :])
            wv = cpool.tile([64, 64], F32)
            nc.scalar.dma_start(wv[:], wv_t[:, :])
            bv = cpool.tile([64, 1], F32)
            nc.scalar.dma_start(bv[:], bv_t[:, :])
            ones_h = cpool.tile([128, 2], BF16)
            nc.vector.memset(ones_h[:], 1.0)

            # ---- fp32 x shard load (only consumed by phase 2; streams through
            # phase 1 + the collective bubble on the sync queues) ----
            xs = xpool.tile([P, N_SHARD], F32R)
            for k in range(N_LDCH):
                sl = slice(k * LDCHUNK, (k + 1) * LDCHUNK)
                nc.sync.dma_start(xs[:, sl], x_t[:, sl])

            # ---- phase 1: G_psum[:,0:128] += xT^T xT ; cols 128:130 = sums ----
            gs = cpool.tile([P, 130], F32)
            with tc.tile_pool(name="gacc", bufs=1, space="PSUM") as gpool:
                # separate PSUM banks: interleaved start/stop accumulation
                # chains sharing one bank clobber each other's has_written
                g_ps = gpool.tile([P, 128], F32, tag="g")
                s_ps1 = gpool.tile([P, 2], F32, tag="s")
                for t in range(N_SLAB):
                    slab = spool.tile([P, SLAB, GCHUNK], BF16)
                    nc.scalar.dma_start(slab[:], xh_t[:, t * SLAB : (t + 1) * SLAB, :])
                    for q in range(SLAB):
                        j = t * SLAB + q
                        nc.tensor.matmul(
                            g_ps[:],
                            lhsT=slab[:, q, :],
                            rhs=slab[:, q, :],
                            start=(j == 0),
                            stop=(j == N_GCH - 1),
                        )
                        nc.tensor.matmul(
                            s_ps1[:],
                            lhsT=slab[:, q, :],
                            rhs=ones_h[:],
                            start=(j == 0),
                            stop=(j == N_GCH - 1),
                        )
                nc.vector.tensor_copy(gs[:, 0:128], g_ps[:])
                nc.vector.tensor_copy(gs[:, 128:130], s_ps1[:])

            # ---- all-reduce [G|s] over the 8 cores (HBM bounce) ----
            cc_in = dram.tile([P, 130], F32)
            cc_out = dram.tile([P, 130], F32)
            nc.scalar.dma_start(cc_in, gs[:])
            nc.gpsimd.collective_compute(
                "AllReduce",
                mybir.AluOpType.add,
                replica_groups=[list(range(NCORES))],
                ins=[cc_in.opt()],
                outs=[cc_out.opt()],
            )
            gsr = cpool.tile([P, 130], F32)
            nc.scalar.dma_start(gsr[:], cc_out)

            # ---- tiny math: S = wqa^T G_aug wka ; softmax ; QT, c ----
            mpool = tc.alloc_tile_pool(name="pmath", bufs=1, space="PSUM")
            # s^T row via PE transpose of the s column
            st_ps = mpool.tile([1, 128], F32, tag="m1")
            nc.tensor.transpose(st_ps[:], gsr[:, 128:129], ident[:])
            st = cpool.tile([1, 128], F32)
            nc.vector.tensor_copy(st[:], st_ps[:])

            ga = []
            for b in range(B):
                g_aug = cpool.tile([65, 65], F32, tag=f"ga{b}", name=f"g_aug{b}")
                cs = slice(b * 64, (b + 1) * 64)
                nc.vector.tensor_copy(g_aug[0:64, 0:64], gsr[cs, cs])
                nc.vector.tensor_copy(g_aug[0:64, 64:65], gsr[cs, 128:129])
                nc.vector.tensor_copy(g_aug[64:65, 0:64], st[:, cs])
                nc.vector.memset(g_aug[64:65, 64:65], float(N_TOTAL))
                ga.append(g_aug)

            # A_b = G_aug_b @ wka  (G_aug symmetric -> lhsT = G_aug)
            s_ps = mpool.tile([P, 64], F32, tag="m2")
            for b in range(B):
                a_ps = mpool.tile([65, 64], F32, tag="m1", name=f"a_ps{b}")
                nc.tensor.matmul(a_ps[:], lhsT=ga[b][:], rhs=wka[:])
                a_sb = cpool.tile([65, 64], F32, tag=f"asb{b}", name=f"a_sb{b}")
                nc.vector.tensor_copy(a_sb[:], a_ps[:])
                # S_b = wqa^T @ A_b   (1/8 scale folded into wqa)
                nc.tensor.matmul(
                    s_ps[b * 64 : (b + 1) * 64, :], lhsT=wqa[:], rhs=a_sb[:]
                )

            # softmax rows (both batches stacked [128, 64])
            negmax = cpool.tile([P, 1], F32)
            nc.vector.reduce_max(
                negmax[:], s_ps[:], axis=mybir.AxisListType.X, negate=True
            )
            expv = cpool.tile([P, 64], F32)
            rowsum = cpool.tile([P, 1], F32)
            nc.scalar.activation(
                expv[:], s_ps[:], mybir.ActivationFunctionType.Exp,
                bias=negmax[:, 0:1], scale=1.0, accum_out=rowsum[:, 0:1],
            )
            rinv = cpool.tile([P, 1], F32)
            nc.vector.reciprocal(rinv[:], rowsum[:])
            attn = cpool.tile([P, 64], F32)
            nc.vector.tensor_scalar_mul(attn[:], expv[:], rinv[:, 0:1])

            # attn^T (one transpose: [128,64] -> [64,128] = [attn0^T | attn1^T])
            at_ps = mpool.tile([64, 128], F32, tag="m1")
            nc.tensor.transpose(at_ps[:], attn[:], ident[:])
            at_sb = cpool.tile([64, 128], F32)
            nc.vector.tensor_copy(at_sb[:], at_ps[:])

            # QT block-diag [128,128]: QT_b = wv^T attn_b^T + I
            qt_ps = mpool.tile([128, 128], F32, tag="m2")
            c_ps = mpool.tile([128, 1], F32, tag="m3")
            for b in range(B):
                cs = slice(b * 64, (b + 1) * 64)
                nc.tensor.matmul(
                    qt_ps[cs, cs], lhsT=wv[:], rhs=at_sb[:, cs],
                    start=True, stop=False,
                )
                nc.tensor.matmul(
                    qt_ps[cs, cs], lhsT=ident[0:64, 0:64], rhs=ident[0:64, 0:64],
                    start=False, stop=True,
                )
                nc.tensor.matmul(c_ps[cs, :], lhsT=at_sb[:, cs], rhs=bv[:])
            qt = cpool.tile([128, 128], F32)
            nc.vector.memset(qt[:], 0.0)
            for b in range(B):
                cs = slice(b * 64, (b + 1) * 64)
                nc.vector.tensor_copy(qt[cs, cs], qt_ps[cs, cs])
            qt_r = cpool.tile([128, 128], F32R)
            nc.vector.tensor_copy(qt_r[:], qt[:])
            cvec = cpool.tile([P, 1], F32)
            nc.vector.tensor_copy(cvec[:], c_ps[:])
            mpool.release()

            # ---- phase 2: out = QT^T x + c  (fp32r single-pass matmuls) ----
            with tc.tile_pool(name="ops", bufs=6, space="PSUM") as oppool:
                for k in range(N_SHARD // OSTORE):
                    osb = opool.tile([P, OSTORE], F32)
                    for h in range(2):
                        sl = slice(k * OSTORE + h * OCHUNK, k * OSTORE + (h + 1) * OCHUNK)
                        o_ps = oppool.tile([P, OCHUNK], F32)
                        nc.tensor.matmul(o_ps[:], lhsT=qt_r[:], rhs=xs[:, sl])
                        oslice = osb[:, h * OCHUNK : (h + 1) * OCHUNK]
                        if h == 0:
                            nc.vector.tensor_scalar_add(oslice, o_ps[:], cvec[:, 0:1])
                        else:
                            nc.scalar.add(oslice, o_ps[:], cvec[:, 0:1])
                    nc.sync.dma_start(
                        out_t[:, k * OSTORE : (k + 1) * OSTORE], osb[:]
                    )

    nc.compile()
    return nc


_cached_nc = None


def kernel(x, wq, bq, wk, bk, wv, bv, _trace=False):
    global _cached_nc
    x = np.ascontiguousarray(np.asarray(x, dtype=np.float32))
    assert x.shape == (B, C, 64, 64, 64)
    xf = x.reshape(P, N_TOTAL)

    wqa = (
        np.concatenate(
            [np.asarray(wq, np.float64), np.asarray(bq, np.float64)[:, None]], axis=1
        ).T
        / 8.0
    ).astype(np.float32)  # [65, 64]
    wka = (
        np.concatenate(
            [np.asarray(wk, np.float64), np.asarray(bk, np.float64)[:, None]], axis=1
        ).T
    ).astype(np.float32)  # [65, 64]
    wv32 = np.ascontiguousarray(np.asarray(wv, np.float32))
    bv32 = np.ascontiguousarray(np.asarray(bv, np.float32).reshape(64, 1))
    ident = np.eye(128, dtype=np.float32)

    in_maps = []
    for i in range(NCORES):
        sl = slice(i * N_SHARD, (i + 1) * N_SHARD)
        xsh = np.ascontiguousarray(xf[:, sl])
        # xh[p, q, c] = x[c, q*128 + p] in fp16 (gram operand, n on partitions)
        xh = np.ascontiguousarray(
            xsh.astype(ml_dtypes.bfloat16).reshape(P, N_GCH, GCHUNK).transpose(2, 1, 0)
        )
        in_maps.append(
            {
                "x": xsh,
                "xh": xh,
                "wqa": wqa,
                "wka": wka,
                "wv": wv32,
                "bv": bv32,
                "ident": ident,
            }
        )

    if _cached_nc is None:
        _cached_nc = build_bass()
    nc = _cached_nc

    res = bass_utils.run_bass_kernel_spmd(
        nc, in_maps, core_ids=list(range(NCORES)), trace=_trace
    )
    kernel._last_results = res

    out = np.empty((P, N_TOTAL), dtype=np.float32)
    for i in range(NCORES):
        out[:, i * N_SHARD : (i + 1) * N_SHARD] = res.results[i]["out"]
    return out.reshape(B, C, 64, 64, 64)


kernel._last_results = None
